# revision 38
# baseline (speedup 1.0000x reference)
"""Contrastive (NT-Xent-style) loss kernel for Trainium2, 8 NeuronCores.

Problem: z1, z2 [16384, 256] fp32.
  h1 = l2norm(z1, axis=1); h2 = l2norm(z2, axis=1)
  sim = h1 @ h2.T                       [N, N]
  between = exp(sim / tau)
  loss = sum_i -log(diag_i / (rowsum_i - diag_i))
       = sum_i [ log(rowsum_i - diag_i) - sim_ii / tau ]

Sharding: z1 rows split across 8 cores (2048 rows each); z2 replicated.

v2 design (vs bf16 baseline):
  * h1/h2 quantized to fp8e4 (x32 scale) and the sim matmul runs in
    DoubleRow (double-pumped fp8) mode: the whole K=256 contraction in
    one PE pass at 2 elems/cycle.
  * The 33.5M-element exp+rowsum stream is split between the ACT engine
    (Exp activation with fused accum_out) and the DVE via a custom
    fused op  body = (((x+A)x+B)x+C)^2, accum += body  which evaluates
    exp(sim/tau)/K (cubic in half-log-domain, squared) in a single 1x
    PSUM pass per tile.  K is folded back in at finalize.
  * sum-of-squares for the row norms uses a custom single-src sq+accum
    DVE op (2x-capable) instead of a 2-input scalar_tensor_tensor.
  * The diagonal (positive-pair) path stays exact fp32.
"""

import numpy as np

# ---- problem constants (hardcoded per contract) ----
N_FULL = 16384
D = 256
TAU = 0.2
N_CORES = 8
P = 128                      # partitions
M_LOC = N_FULL // N_CORES    # 2048 z1 rows per core
M_TILES = M_LOC // P         # 16
G = 8                        # z2 row groups per core
G_ROWS = N_FULL // G         # 2048 z2 rows per group
G_TILES = G_ROWS // P        # 16
NSUB = 4                     # 512-wide matmul sub-chunks per psum tile
PSUM_N = NSUB * 512          # 2048
KD = 2                       # contraction split: 256 = 2 x 128
RSQRT_MAGIC = 0x5F3759DF
RSQRT_MAGIC32 = 0x5F3759DF + (5 << 23)  # seeds 32/sqrt(x)

FP8_SCALE = 32.0             # h rows scaled by 32 before e4m3 quantize
S2 = FP8_SCALE * FP8_SCALE   # 1024: psum raw = sim * S2
ACT_SCALE = 1.0 / (S2 * TAU)

# cubic fit of exp(t) on t in [-0.95, 0.95]:  d*(t^3 + a t^2 + b t + c)
# (see transcript: minimax-ish relative fit, max rel err 0.64%).
# body(x) = (x^3 + A x^2 + B x + C), x = raw psum value = t/m,
# m = 1/(2*S2*TAU);  body^2 = exp(sim/tau) / EXP_K.
_D3, _D2, _D1, _D0 = (0.15713039, 0.53074203, 1.00816094, 0.99775348)
_M = 1.0 / (2.0 * S2 * TAU)
EXP_A = (_D2 / _D3) / _M
EXP_B = (_D1 / _D3) / _M**2
EXP_C = (_D0 / _D3) / _M**3
EXP_K = (_D3 * _M**3) ** 2

# m-tiles whose exp+rowsum is drained by the DVE custom op (rest: ACT)
DVE_MS = frozenset((2, 5, 8, 11, 14))

_CACHE = {}


def _register_dve_ops():
    """Register the two custom DVE ops (idempotent). Returns (exp_op, sq_op)."""
    if "dve_ops" in _CACHE:
        return _CACHE["dve_ops"]
    from operator import add

    import concourse.dve_ops as dve_ops
    from concourse.dve_spec import Spec, Src0, C0, C1, C2, Zero, lower, sq
    from concourse.dve_table_gen import dve_ver_for
    from concourse.dve_uop import DveOpSpec

    def make_op(name, spec, perf_en=None):
        existing = [op for op in dve_ops.OPS if op.name == name]
        if existing:
            return existing[0]
        row = dve_ops._CUSTOM_DVE_ROW_BASE + len(dve_ops.OPS)
        dve_ops._SUB_OPCODE_FOR_NAME[name] = row
        shas = {}
        for ver in ("v3", "v4"):
            try:
                uops = lower(spec, ver=ver)
            except Exception:
                continue
            from concourse.dve_spec import _has_src1

            shas[ver] = DveOpSpec(
                name=name, opcode=row, uops=uops, rd1_en=_has_src1(spec)
            ).sha(ver)
        op = dve_ops.DveOp(
            name, spec, subdim=False, uops_sha=shas, perf_en=perf_en or {}
        )
        dve_ops.OPS.append(op)
        dve_ops.CUSTOM_DVE_SPECS[name] = spec
        return op

    def _exp_ref(in0, in1, c0, c1, c2):
        x = in0.astype(np.float32)
        b = ((((x + c0) * x + c1) * x + c2) ** 2).astype(np.float32)
        return b, b.reshape(b.shape[0], -1).sum(axis=-1, keepdims=True)

    exp_spec = Spec(
        body=sq(((Src0 + C0) * Src0 + C1) * Src0 + C2),
        accum=add,
        accum_init=Zero,
        reference=_exp_ref,
    )
    exp_op = make_op("EXP3SQ_ACC_ANT", exp_spec)

    def _sq_ref(in0, in1, c0, c1, c2):
        x = in0.astype(np.float32)
        b = (x * x).astype(np.float32)
        return b, b.reshape(b.shape[0], -1).sum(axis=-1, keepdims=True)

    sq_spec = Spec(
        body=sq(Src0),
        accum=add,
        accum_init=Zero,
        reference=_sq_ref,
    )
    sq_op = make_op("SQACC_ANT", sq_spec, perf_en={"v3": True})

    _CACHE["dve_ops"] = (exp_op, sq_op)
    return exp_op, sq_op


def _build_nc():
    from contextlib import ExitStack

    import concourse.bacc as bacc
    import concourse.tile as tile
    from concourse import mybir
    from concourse.masks import make_identity

    exp_op, sq_op = _register_dve_ops()

    AF = mybir.ActivationFunctionType
    ALU = mybir.AluOpType
    FP32 = mybir.dt.float32
    INT32 = mybir.dt.int32
    BF16 = mybir.dt.bfloat16
    FP8 = mybir.dt.float8e4
    DR = mybir.MatmulPerfMode.DoubleRow

    nc = bacc.Bacc("TRN2", target_bir_lowering=False, debug=False)

    z1 = nc.dram_tensor("z1", [M_LOC, D], FP32, kind="ExternalInput").ap()
    z2 = nc.dram_tensor("z2", [N_FULL, D], BF16, kind="ExternalInput").ap()
    z2d = nc.dram_tensor("z2d", [M_LOC, D], FP32, kind="ExternalInput").ap()
    out_parts = nc.dram_tensor(
        "loss_parts", [P, M_TILES], FP32, kind="ExternalOutput"
    ).ap()

    with tile.TileContext(nc) as tc, ExitStack() as ctx:
        pz1 = ctx.enter_context(tc.tile_pool(name="z1p", bufs=1))
        pz2d = ctx.enter_context(tc.tile_pool(name="z2dp", bufs=1))
        pzg = ctx.enter_context(tc.tile_pool(name="zgp", bufs=2))
        ph1 = ctx.enter_context(tc.tile_pool(name="h1p", bufs=1))
        ph2 = ctx.enter_context(tc.tile_pool(name="h2p", bufs=2))
        pid = ctx.enter_context(tc.tile_pool(name="idp", bufs=1))
        pscr = ctx.enter_context(tc.tile_pool(name="scrp", bufs=4))
        phq = ctx.enter_context(tc.tile_pool(name="hqp", bufs=3))
        pst = ctx.enter_context(tc.tile_pool(name="stats", bufs=1))
        pgst = ctx.enter_context(tc.tile_pool(name="gstats", bufs=2))
        ppsum = ctx.enter_context(tc.tile_pool(name="psump", bufs=2, space="PSUM"))

        ident = pid.tile([P, P], BF16, tag="ident")
        make_identity(nc, ident[:])

        def sumsq(dst, a):
            """dst[:,:1] = sum over free dim of a*a (custom DVE sq+accum)."""
            s = pscr.tile([P, D], FP32, tag="scr")
            nc.vector._custom_dve(sq_op, out=s[:], in0=a, accum_out=dst)

        def rsqrt32_dve(ssq, pool, tag, w):
            """32/sqrt(ssq) entirely on DVE: bit-trick seed + 2 Newton steps
            (the x32 fp8 scale is folded into the seed and Newton constant)."""
            y = pool.tile([P, w], FP32, tag=tag)
            t1 = pool.tile([P, w], FP32, tag=tag + "_t1")
            t2 = pool.tile([P, w], FP32, tag=tag + "_t2")
            yi = y[:].bitcast(INT32)
            nc.vector.tensor_scalar(
                yi, ssq.bitcast(INT32), 1, None, ALU.logical_shift_right
            )
            nc.vector.tensor_scalar(yi, yi, -1, RSQRT_MAGIC32, ALU.mult, ALU.add)
            for _ in range(2):
                nc.vector.tensor_mul(t1[:], y[:], y[:])
                nc.vector.scalar_tensor_tensor(
                    t2[:], in0=ssq, scalar=-0.5 / (FP8_SCALE * FP8_SCALE),
                    in1=t1[:], op0=ALU.mult, op1=ALU.mult,
                )
                nc.vector.tensor_scalar(t2[:], t2[:], 1.5, None, ALU.add)
                nc.vector.tensor_mul(y[:], y[:], t2[:])
            return y

        def sq_chunk(zt, sq_scr, t0, nt=4):
            """sq_scr[:, t0:t0+nt] = zt^2 (bf16 2x tensor_tensor)."""
            nc.vector.tensor_mul(
                sq_scr[:, t0 : t0 + nt, :], zt[:, t0 : t0 + nt, :],
                zt[:, t0 : t0 + nt, :],
            )

        def red_chunk(sq_scr, ssq, t0, nt=4):
            nc.vector.tensor_reduce(
                ssq[:, t0 : t0 + nt], sq_scr[:, t0 : t0 + nt, :],
                axis=mybir.AxisListType.X, op=ALU.add,
            )

        def quant_chunk(zt, rn32, hq, t0, nt=4):
            """hq[:,t0:t0+nt] fp8 = zt * rn32 (per-row-tile scale, bcast)."""
            nc.vector.scalar_tensor_tensor(
                hq[:, t0 : t0 + nt, :], in0=zt[:, t0 : t0 + nt, :], scalar=1.0,
                in1=rn32[:, t0 : t0 + nt]
                .rearrange("p (t o) -> p t o", o=1)
                .broadcast_to([P, nt, D]),
                op0=ALU.mult, op1=ALU.mult,
            )

        def group_sumsq(zt, ssq, sq_scr):
            """prologue-only: batched square + reduce."""
            nc.vector.tensor_mul(sq_scr[:], zt[:], zt[:])
            nc.vector.tensor_reduce(
                ssq[:], sq_scr[:], axis=mybir.AxisListType.X, op=ALU.add
            )

        def group_quant(zt, rn32, hq):
            """prologue-only: hq fp8 = zt * rn32 (broadcast scale)."""
            nc.vector.scalar_tensor_tensor(
                hq[:], in0=zt[:], scalar=1.0,
                in1=rn32[:].rearrange("p (t o) -> p t o", o=1).broadcast_to(
                    [P, G_TILES, D]
                ),
                op0=ALU.mult, op1=ALU.mult,
            )

        def xpose_burst2(hq, s0, dst, t0, n=8):
            """PE-transpose fp8 row-tiles hq[:, s0+j, :] as bf16 byte-pairs
            into dst[:, t0*P:...] (bf16 [P, N]); the transpose is a pure
            permutation so the packed (fp8 d=2c, fp8 d=2c+1) pairs land
            intact at contraction-partition c."""
            pt = ppsum.tile([P, n, P], BF16, tag="ps")
            for j in range(n):
                nc.tensor.transpose(
                    pt[:, j, :], hq[:, s0 + j, :].bitcast(BF16), ident[:]
                )
            nc.vector.tensor_copy(
                dst[:, t0 * P : (t0 + n) * P].bitcast(INT32),
                pt[:, :, :].bitcast(INT32),
            )

        # ---------- prologue: z1 / group-0 prep ----------
        def load_group(g):
            zt = pzg.tile([P, G_TILES, D], BF16, tag="zgt")
            nc.sync.dma_start(
                zt[:],
                z2[g * G_ROWS : (g + 1) * G_ROWS, :].rearrange(
                    "(t p) d -> p t d", p=P
                ),
            )
            return zt

        # group-0 slice in halves so its first h2T columns are ready early
        zgt_cur = pzg.tile([P, G_TILES, D], BF16, tag="zgt")
        HT = G_TILES // 2
        nc.sync.dma_start(
            zgt_cur[:, :HT, :],
            z2[0 : HT * P, :].rearrange("(t p) d -> p t d", p=P),
        )
        z1t = pz1.tile([P, M_TILES, D], FP32, tag="z1t")
        nc.sync.dma_start(z1t[:], z1.rearrange("(t p) d -> p t d", p=P))
        nc.sync.dma_start(
            zgt_cur[:, HT:, :],
            z2[HT * P : G_ROWS, :].rearrange("(t p) d -> p t d", p=P),
        )

        ssq1 = pst.tile([P, M_TILES], FP32, tag="ssq1")
        ssq2d = pst.tile([P, M_TILES], FP32, tag="ssq2d")
        d_raw = pst.tile([P, M_TILES], FP32, tag="d_raw")
        # z1 sum-of-squares on ACT (idle in prologue), in halves so the
        # first h1Tp half is ready early
        z1sq = pz1.tile([P, M_TILES, D], FP32, tag="z1sq")
        H = M_TILES // 2
        for h in range(2):
            nc.scalar.activation(
                z1sq[:, h * H : (h + 1) * H, :].rearrange("p t d -> p (t d)"),
                z1t[:, h * H : (h + 1) * H, :].rearrange("p t d -> p (t d)"),
                AF.Square,
            )
        def pairs(hT):
            """fp8 DoubleRow view [P, 2, N] of a packed-pairs bf16 tile."""
            return hT[:].bitcast(FP8).rearrange("p (j k) -> p k j", k=2)

        # group 0 prep on DVE in halves (parallel with ACT's z1 squares)
        ssqg = pgst.tile([P, G_TILES], FP32, tag="ssqg")
        sq_scr = pgst.tile([P, G_TILES, D], BF16, tag="sq_scr")
        hq_cur = phq.tile([P, G_TILES, D], FP8, tag="hq")
        h2T_cur = ph2.tile([P, G_ROWS], BF16, tag="h2T")
        for hh in range(2):
            sq_chunk(zgt_cur, sq_scr, hh * HT, HT)
            red_chunk(sq_scr, ssqg, hh * HT, HT)
            rng32_h = rsqrt32_dve(
                ssqg[:, hh * HT : (hh + 1) * HT], pgst, f"rng32{hh}", HT
            )
            for t in range(HT):
                nc.vector.tensor_scalar(
                    hq_cur[:, hh * HT + t, :], zgt_cur[:, hh * HT + t, :],
                    rng32_h[:, t : t + 1], None, ALU.mult,
                )
            xpose_burst2(hq_cur, hh * HT, h2T_cur, hh * HT)

        # z1 path in halves (reduce/rsqrt/quant/xpose/deint per half) so the
        # first matmuls can start before the second half is prepped
        h1T = ph1.tile([P, M_LOC], BF16, tag="h1T")
        hq1 = phq.tile([P, M_TILES, D], FP8, tag="hq")
        h1Tp = ph1.tile([P, KD, M_LOC], FP8, tag="h1Tp")
        HL = M_LOC // 2
        rn1s_halves = []
        for h in range(2):
            red_chunk(z1sq, ssq1, h * H, H)
            rn1s_h = rsqrt32_dve(
                ssq1[:, h * H : (h + 1) * H], pst, f"rn1s{h}", H
            )
            rn1s_halves.append(rn1s_h)
            for t in range(H):
                nc.vector.tensor_scalar(
                    hq1[:, h * H + t, :], z1t[:, h * H + t, :],
                    rn1s_h[:, t : t + 1], None, ALU.mult,
                )
            xpose_burst2(hq1, h * H, h1T, h * H)
            for k in range(KD):
                nc.vector.tensor_copy(
                    h1Tp[:, k, h * HL : (h + 1) * HL],
                    pairs(h1T)[:, k, h * HL : (h + 1) * HL],
                )
        rn1s_lo, rn1s_hi = rn1s_halves

        parts_act = pst.tile([P, M_TILES, G], FP32, tag="parts_act")
        parts_dve = pst.tile([P, M_TILES, G], FP32, tag="parts_dve")
        nc.gpsimd.memset(parts_act[:], 0.0)
        nc.gpsimd.memset(parts_dve[:], 0.0)

        # ---------- main loop over z2 groups ----------
        for g in range(G):
            nxt = {}
            for m in range(M_TILES):
                ps = ppsum.tile([P, PSUM_N], FP32, tag="ps")
                h2p = pairs(h2T_cur)
                for sub in range(NSUB):
                    nc.tensor.matmul(
                        ps[:, sub * 512 : (sub + 1) * 512],
                        h1Tp[:, :, m * P : (m + 1) * P],
                        h2p[:, :, sub * 512 : (sub + 1) * 512],
                        start=True,
                        stop=True,
                        perf_mode=DR,
                    )
                if m in DVE_MS:
                    nc.vector._custom_dve(
                        exp_op,
                        out=ps[:],
                        in0=ps[:],
                        s0=EXP_A,
                        s1=EXP_B,
                        imm2=EXP_C,
                        accum_out=parts_dve[:, m, g : g + 1],
                    )
                else:
                    nc.scalar.activation(
                        ps[:], ps[:], AF.Exp, scale=ACT_SCALE,
                        accum_out=parts_act[:, m, g : g + 1],
                    )
                if g + 1 < G:
                    if m == 0:
                        nxt["zt"] = load_group(g + 1)
                        sq_nxt = pgst.tile([P, G_TILES, D], BF16, tag="sq_scr")
                        ssq_nxt = pgst.tile([P, G_TILES], FP32, tag="ssqg")
                        hq_nxt = phq.tile([P, G_TILES, D], FP8, tag="hq")
                        h2T_nxt = ph2.tile([P, G_ROWS], BF16, tag="h2T")
                        nxt["sq"], nxt["ssq"] = sq_nxt, ssq_nxt
                        nxt["hq"], nxt["h2T"] = hq_nxt, h2T_nxt
                    elif m <= 8:
                        for t in range(2 * (m - 1), 2 * m):
                            if t % 2 == 0:
                                s = pscr.tile([P, D], FP32, tag="scr")
                                nc.scalar.activation(
                                    s[:], nxt["zt"][:, t, :], AF.Square,
                                    accum_out=nxt["ssq"][:, t : t + 1],
                                )
                            else:
                                sumsq(
                                    nxt["ssq"][:, t : t + 1], nxt["zt"][:, t, :]
                                )
                    elif m == 9:
                        nxt["rn32"] = rsqrt32_dve(
                            nxt["ssq"][:], pgst, "rng32", G_TILES
                        )
                    elif m in (10, 11, 12):
                        t0 = {10: 0, 11: 5, 12: 10}[m]
                        nt = {10: 5, 11: 5, 12: 6}[m]
                        for t in range(t0, t0 + nt):
                            nc.vector.tensor_scalar(
                                nxt["hq"][:, t, :], nxt["zt"][:, t, :],
                                nxt["rn32"][:, t : t + 1], None, ALU.mult,
                            )
                    elif m == 13:
                        xpose_burst2(nxt["hq"], 0, nxt["h2T"], 0, n=16)
                else:
                    # last group: the diagonal (positive-pair) path
                    if m == 0:
                        z2dt = pz2d.tile([P, M_TILES, D], FP32, tag="z2dt")
                        nc.sync.dma_start(
                            z2dt[:], z2d.rearrange("(t p) d -> p t d", p=P)
                        )
                    elif 6 <= m <= 9:
                        for t in range(4 * (m - 6), 4 * (m - 5)):
                            sumsq(ssq2d[:, t : t + 1], z2dt[:, t, :])
                    elif m == 10:
                        rn2d32 = rsqrt32_dve(ssq2d[:], pst, "rn2d32", M_TILES)
                    elif 11 <= m <= 14:
                        for mm in range(4 * (m - 11), 4 * (m - 10)):
                            s = pscr.tile([P, D], FP32, tag="scr")
                            nc.vector.scalar_tensor_tensor(
                                s[:],
                                in0=z1t[:, mm, :],
                                scalar=1.0,
                                in1=z2dt[:, mm, :],
                                op0=ALU.mult,
                                op1=ALU.mult,
                                accum_out=d_raw[:, mm : mm + 1],
                            )
            if g + 1 < G:
                zgt_cur = nxt["zt"]
                h2T_cur = nxt["h2T"]

        # ---------- finalize ----------
        st = pst.tile([P, M_TILES], FP32, tag="st")
        nc.vector.tensor_mul(st[:, :H], d_raw[:, :H], rn1s_lo[:])
        nc.vector.tensor_mul(st[:, H:], d_raw[:, H:], rn1s_hi[:])
        nc.vector.tensor_mul(st[:], st[:], rn2d32[:])
        nc.vector.tensor_scalar(st[:], st[:], 1.0 / (TAU * S2), None, ALU.mult)
        dex = pst.tile([P, M_TILES], FP32, tag="dex")
        nc.scalar.activation(dex[:], st[:], AF.Exp)
        rows_a = pst.tile([P, M_TILES], FP32, tag="rows_a")
        nc.vector.tensor_reduce(
            rows_a[:], parts_act[:], axis=mybir.AxisListType.X, op=ALU.add
        )
        rows_d = pst.tile([P, M_TILES], FP32, tag="rows_d")
        nc.vector.tensor_reduce(
            rows_d[:], parts_dve[:], axis=mybir.AxisListType.X, op=ALU.add
        )
        rows = pst.tile([P, M_TILES], FP32, tag="rows")
        nc.vector.scalar_tensor_tensor(
            rows[:], in0=rows_d[:], scalar=EXP_K, in1=rows_a[:],
            op0=ALU.mult, op1=ALU.add,
        )
        neg = pst.tile([P, M_TILES], FP32, tag="neg")
        nc.vector.tensor_sub(neg[:], rows[:], dex[:])
        lneg = pst.tile([P, M_TILES], FP32, tag="lneg")
        nc.scalar.activation(lneg[:], neg[:], AF.Ln)
        lp = pst.tile([P, M_TILES], FP32, tag="lp")
        nc.vector.tensor_sub(lp[:], lneg[:], st[:])
        nc.sync.dma_start(out_parts, lp[:])

    nc.compile()
    return nc


def get_nc():
    if "nc" not in _CACHE:
        _CACHE["nc"] = _build_nc()
    return _CACHE["nc"]


def make_in_maps(z1, z2):
    import ml_dtypes

    z1 = np.ascontiguousarray(np.asarray(z1, dtype=np.float32))
    z2 = np.ascontiguousarray(np.asarray(z2, dtype=np.float32))
    z2h = np.ascontiguousarray(z2.astype(ml_dtypes.bfloat16))
    in_maps = []
    for c in range(N_CORES):
        blk = slice(c * M_LOC, (c + 1) * M_LOC)
        in_maps.append({"z1": z1[blk], "z2": z2h, "z2d": z2[blk]})
    return in_maps


def kernel(z1, z2):
    from concourse.bass_utils import run_bass_kernel_spmd

    nc = get_nc()
    res = run_bass_kernel_spmd(nc, make_in_maps(z1, z2), core_ids=list(range(N_CORES)))
    total = 0.0
    for c in range(N_CORES):
        total += res.results[c]["loss_parts"].astype(np.float64).sum()
    return np.float32(total)


# revision 39
# speedup vs baseline: 1.0566x; 1.0566x over previous
"""Contrastive (NT-Xent-style) loss kernel for Trainium2, 8 NeuronCores.

Problem: z1, z2 [16384, 256] fp32.
  h1 = l2norm(z1, axis=1); h2 = l2norm(z2, axis=1)
  sim = h1 @ h2.T                       [N, N]
  between = exp(sim / tau)
  loss = sum_i -log(diag_i / (rowsum_i - diag_i))
       = sum_i [ log(rowsum_i - diag_i) - sim_ii / tau ]

Sharding: z1 rows split across 8 cores (2048 rows each); z2 replicated.

v2 design (vs bf16 baseline):
  * h1/h2 quantized to fp8e4 (x32 scale) and the sim matmul runs in
    DoubleRow (double-pumped fp8) mode: the whole K=256 contraction in
    one PE pass at 2 elems/cycle.
  * The 33.5M-element exp+rowsum stream is split between the ACT engine
    (Exp activation with fused accum_out) and the DVE via a custom
    fused op  body = (((x+A)x+B)x+C)^2, accum += body  which evaluates
    exp(sim/tau)/K (cubic in half-log-domain, squared) in a single 1x
    PSUM pass per tile.  K is folded back in at finalize.
  * sum-of-squares for the row norms uses a custom single-src sq+accum
    DVE op (2x-capable) instead of a 2-input scalar_tensor_tensor.
  * The diagonal (positive-pair) path stays exact fp32.
"""

import numpy as np

# ---- problem constants (hardcoded per contract) ----
N_FULL = 16384
D = 256
TAU = 0.2
N_CORES = 8
P = 128                      # partitions
M_LOC = N_FULL // N_CORES    # 2048 z1 rows per core
M_TILES = M_LOC // P         # 16
G = 8                        # z2 row groups per core
G_ROWS = N_FULL // G         # 2048 z2 rows per group
G_TILES = G_ROWS // P        # 16
NSUB = 4                     # 512-wide matmul sub-chunks per psum tile
PSUM_N = NSUB * 512          # 2048
KD = 2                       # contraction split: 256 = 2 x 128
RSQRT_MAGIC = 0x5F3759DF
RSQRT_MAGIC32 = 0x5F3759DF + (5 << 23)  # seeds 32/sqrt(x)

FP8_SCALE = 32.0             # h rows scaled by 32 before e4m3 quantize
S2 = FP8_SCALE * FP8_SCALE   # 1024: psum raw = sim * S2
ACT_SCALE = 1.0 / (S2 * TAU)

# cubic fit of exp(t) on t in [-0.95, 0.95]:  d*(t^3 + a t^2 + b t + c)
# (see transcript: minimax-ish relative fit, max rel err 0.64%).
# body(x) = (x^3 + A x^2 + B x + C), x = raw psum value = t/m,
# m = 1/(2*S2*TAU);  body^2 = exp(sim/tau) / EXP_K.
_D3, _D2, _D1, _D0 = (0.15713039, 0.53074203, 1.00816094, 0.99775348)
_M = 1.0 / (2.0 * S2 * TAU)
EXP_A = (_D2 / _D3) / _M
EXP_B = (_D1 / _D3) / _M**2
EXP_C = (_D0 / _D3) / _M**3
EXP_K = (_D3 * _M**3) ** 2

# m-tiles whose exp+rowsum is drained by the DVE custom op (rest: ACT)
DVE_MS = frozenset((2, 5, 8, 11, 14))

_CACHE = {}


def _register_dve_ops():
    """Register the two custom DVE ops (idempotent). Returns (exp_op, sq_op)."""
    if "dve_ops" in _CACHE:
        return _CACHE["dve_ops"]
    from operator import add

    import concourse.dve_ops as dve_ops
    from concourse.dve_spec import Spec, Src0, C0, C1, C2, Zero, lower, sq
    from concourse.dve_table_gen import dve_ver_for
    from concourse.dve_uop import DveOpSpec

    def make_op(name, spec, perf_en=None):
        existing = [op for op in dve_ops.OPS if op.name == name]
        if existing:
            return existing[0]
        row = dve_ops._CUSTOM_DVE_ROW_BASE + len(dve_ops.OPS)
        dve_ops._SUB_OPCODE_FOR_NAME[name] = row
        shas = {}
        for ver in ("v3", "v4"):
            try:
                uops = lower(spec, ver=ver)
            except Exception:
                continue
            from concourse.dve_spec import _has_src1

            shas[ver] = DveOpSpec(
                name=name, opcode=row, uops=uops, rd1_en=_has_src1(spec)
            ).sha(ver)
        op = dve_ops.DveOp(
            name, spec, subdim=False, uops_sha=shas, perf_en=perf_en or {}
        )
        dve_ops.OPS.append(op)
        dve_ops.CUSTOM_DVE_SPECS[name] = spec
        return op

    def _exp_ref(in0, in1, c0, c1, c2):
        x = in0.astype(np.float32)
        b = ((((x + c0) * x + c1) * x + c2) ** 2).astype(np.float32)
        return b, b.reshape(b.shape[0], -1).sum(axis=-1, keepdims=True)

    exp_spec = Spec(
        body=sq(((Src0 + C0) * Src0 + C1) * Src0 + C2),
        accum=add,
        accum_init=Zero,
        reference=_exp_ref,
    )
    exp_op = make_op("EXP3SQ_ACC_ANT", exp_spec)

    def _sq_ref(in0, in1, c0, c1, c2):
        x = in0.astype(np.float32)
        b = (x * x).astype(np.float32)
        return b, b.reshape(b.shape[0], -1).sum(axis=-1, keepdims=True)

    sq_spec = Spec(
        body=sq(Src0),
        accum=add,
        accum_init=Zero,
        reference=_sq_ref,
    )
    sq_op = make_op("SQACC_ANT", sq_spec, perf_en={"v3": True})

    _CACHE["dve_ops"] = (exp_op, sq_op)
    return exp_op, sq_op


def _build_nc():
    from contextlib import ExitStack

    import concourse.bacc as bacc
    import concourse.tile as tile
    from concourse import mybir
    from concourse.masks import make_identity

    exp_op, sq_op = _register_dve_ops()

    AF = mybir.ActivationFunctionType
    ALU = mybir.AluOpType
    FP32 = mybir.dt.float32
    INT32 = mybir.dt.int32
    BF16 = mybir.dt.bfloat16
    FP8 = mybir.dt.float8e4
    DR = mybir.MatmulPerfMode.DoubleRow

    nc = bacc.Bacc("TRN2", target_bir_lowering=False, debug=False)

    z1 = nc.dram_tensor("z1", [M_LOC, D], FP32, kind="ExternalInput").ap()
    z2 = nc.dram_tensor("z2", [N_FULL, D], BF16, kind="ExternalInput").ap()
    z2d = nc.dram_tensor("z2d", [M_LOC, D], FP32, kind="ExternalInput").ap()
    out_parts = nc.dram_tensor(
        "loss_parts", [P, M_TILES], FP32, kind="ExternalOutput"
    ).ap()

    with tile.TileContext(nc) as tc, ExitStack() as ctx:
        pz1 = ctx.enter_context(tc.tile_pool(name="z1p", bufs=1))
        pz2d = ctx.enter_context(tc.tile_pool(name="z2dp", bufs=1))
        pzg = ctx.enter_context(tc.tile_pool(name="zgp", bufs=2))
        ph1 = ctx.enter_context(tc.tile_pool(name="h1p", bufs=1))
        ph2 = ctx.enter_context(tc.tile_pool(name="h2p", bufs=2))
        pid = ctx.enter_context(tc.tile_pool(name="idp", bufs=1))
        pscr = ctx.enter_context(tc.tile_pool(name="scrp", bufs=4))
        phq = ctx.enter_context(tc.tile_pool(name="hqp", bufs=3))
        pst = ctx.enter_context(tc.tile_pool(name="stats", bufs=1))
        pgst = ctx.enter_context(tc.tile_pool(name="gstats", bufs=2))
        ppsum = ctx.enter_context(tc.tile_pool(name="psump", bufs=2, space="PSUM"))

        ident = pid.tile([P, P], BF16, tag="ident")
        make_identity(nc, ident[:])

        def sumsq(dst, a):
            """dst[:,:1] = sum over free dim of a*a (custom DVE sq+accum)."""
            s = pscr.tile([P, D], FP32, tag="scr")
            nc.vector._custom_dve(sq_op, out=s[:], in0=a, accum_out=dst)

        def rsqrt32_dve(ssq, pool, tag, w):
            """32/sqrt(ssq) entirely on DVE: bit-trick seed + 2 Newton steps
            (the x32 fp8 scale is folded into the seed and Newton constant)."""
            y = pool.tile([P, w], FP32, tag=tag)
            t1 = pool.tile([P, w], FP32, tag=tag + "_t1")
            t2 = pool.tile([P, w], FP32, tag=tag + "_t2")
            yi = y[:].bitcast(INT32)
            nc.vector.tensor_scalar(
                yi, ssq.bitcast(INT32), 1, None, ALU.logical_shift_right
            )
            nc.vector.tensor_scalar(yi, yi, -1, RSQRT_MAGIC32, ALU.mult, ALU.add)
            for _ in range(2):
                nc.vector.tensor_mul(t1[:], y[:], y[:])
                nc.vector.scalar_tensor_tensor(
                    t2[:], in0=ssq, scalar=-0.5 / (FP8_SCALE * FP8_SCALE),
                    in1=t1[:], op0=ALU.mult, op1=ALU.mult,
                )
                nc.vector.tensor_scalar(t2[:], t2[:], 1.5, None, ALU.add)
                nc.vector.tensor_mul(y[:], y[:], t2[:])
            return y

        def sq_chunk(zt, sq_scr, t0, nt=4):
            """sq_scr[:, t0:t0+nt] = zt^2 (bf16 2x tensor_tensor)."""
            nc.vector.tensor_mul(
                sq_scr[:, t0 : t0 + nt, :], zt[:, t0 : t0 + nt, :],
                zt[:, t0 : t0 + nt, :],
            )

        def red_chunk(sq_scr, ssq, t0, nt=4):
            nc.vector.tensor_reduce(
                ssq[:, t0 : t0 + nt], sq_scr[:, t0 : t0 + nt, :],
                axis=mybir.AxisListType.X, op=ALU.add,
            )

        def quant_chunk(zt, rn32, hq, t0, nt=4):
            """hq[:,t0:t0+nt] fp8 = zt * rn32 (per-row-tile scale, bcast)."""
            nc.vector.scalar_tensor_tensor(
                hq[:, t0 : t0 + nt, :], in0=zt[:, t0 : t0 + nt, :], scalar=1.0,
                in1=rn32[:, t0 : t0 + nt]
                .rearrange("p (t o) -> p t o", o=1)
                .broadcast_to([P, nt, D]),
                op0=ALU.mult, op1=ALU.mult,
            )

        def group_sumsq(zt, ssq, sq_scr):
            """prologue-only: batched square + reduce."""
            nc.vector.tensor_mul(sq_scr[:], zt[:], zt[:])
            nc.vector.tensor_reduce(
                ssq[:], sq_scr[:], axis=mybir.AxisListType.X, op=ALU.add
            )

        def group_quant(zt, rn32, hq):
            """prologue-only: hq fp8 = zt * rn32 (broadcast scale)."""
            nc.vector.scalar_tensor_tensor(
                hq[:], in0=zt[:], scalar=1.0,
                in1=rn32[:].rearrange("p (t o) -> p t o", o=1).broadcast_to(
                    [P, G_TILES, D]
                ),
                op0=ALU.mult, op1=ALU.mult,
            )

        def xpose_burst2(hq, s0, dst, t0, n=8):
            """PE-transpose fp8 row-tiles hq[:, s0+j, :] as bf16 byte-pairs
            into dst[:, t0*P:...] (bf16 [P, N]); the transpose is a pure
            permutation so the packed (fp8 d=2c, fp8 d=2c+1) pairs land
            intact at contraction-partition c."""
            pt = ppsum.tile([P, n, P], BF16, tag="ps")
            for j in range(n):
                nc.tensor.transpose(
                    pt[:, j, :], hq[:, s0 + j, :].bitcast(BF16), ident[:]
                )
            nc.vector.tensor_copy(
                dst[:, t0 * P : (t0 + n) * P].bitcast(INT32),
                pt[:, :, :].bitcast(INT32),
            )

        # ---------- prologue: z1 / group-0 prep ----------
        def load_group(g):
            zt = pzg.tile([P, G_TILES, D], BF16, tag="zgt")
            nc.sync.dma_start(
                zt[:],
                z2[g * G_ROWS : (g + 1) * G_ROWS, :].rearrange(
                    "(t p) d -> p t d", p=P
                ),
            )
            return zt

        zgt_cur = load_group(0)
        z1t = pz1.tile([P, M_TILES, D], FP32, tag="z1t")
        nc.sync.dma_start(z1t[:], z1.rearrange("(t p) d -> p t d", p=P))

        ssq1 = pst.tile([P, M_TILES], FP32, tag="ssq1")
        ssq2d = pst.tile([P, M_TILES], FP32, tag="ssq2d")
        d_raw = pst.tile([P, M_TILES], FP32, tag="d_raw")
        # z1 sum-of-squares on ACT (idle in prologue), in halves so the
        # first h1Tp half is ready early
        z1sq = pz1.tile([P, M_TILES, D], FP32, tag="z1sq")
        H = M_TILES // 2
        for h in range(2):
            nc.scalar.activation(
                z1sq[:, h * H : (h + 1) * H, :].rearrange("p t d -> p (t d)"),
                z1t[:, h * H : (h + 1) * H, :].rearrange("p t d -> p (t d)"),
                AF.Square,
            )
        def pairs(hT):
            """fp8 DoubleRow view [P, 2, N] of a packed-pairs bf16 tile."""
            return hT[:].bitcast(FP8).rearrange("p (j k) -> p k j", k=2)

        # group 0 prep on DVE (runs in parallel with ACT's z1 square)
        ssqg = pgst.tile([P, G_TILES], FP32, tag="ssqg")
        sq_scr = pgst.tile([P, G_TILES, D], BF16, tag="sq_scr")
        group_sumsq(zgt_cur, ssqg, sq_scr)
        rng32 = rsqrt32_dve(ssqg[:], pgst, "rng32", G_TILES)
        hq_cur = phq.tile([P, G_TILES, D], FP8, tag="hq")
        group_quant(zgt_cur, rng32, hq_cur)
        h2T_cur = ph2.tile([P, G_ROWS], BF16, tag="h2T")
        xpose_burst2(hq_cur, 0, h2T_cur, 0)
        xpose_burst2(hq_cur, 8, h2T_cur, 8)

        # z1 path in halves (reduce/rsqrt/quant/xpose/deint per half) so the
        # first matmuls can start before the second half is prepped
        h1T = ph1.tile([P, M_LOC], BF16, tag="h1T")
        hq1 = phq.tile([P, M_TILES, D], FP8, tag="hq")
        h1Tp = ph1.tile([P, KD, M_LOC], FP8, tag="h1Tp")
        HL = M_LOC // 2
        rn1s_halves = []
        for h in range(2):
            red_chunk(z1sq, ssq1, h * H, H)
            rn1s_h = rsqrt32_dve(
                ssq1[:, h * H : (h + 1) * H], pst, f"rn1s{h}", H
            )
            rn1s_halves.append(rn1s_h)
            for t in range(H):
                nc.vector.tensor_scalar(
                    hq1[:, h * H + t, :], z1t[:, h * H + t, :],
                    rn1s_h[:, t : t + 1], None, ALU.mult,
                )
            xpose_burst2(hq1, h * H, h1T, h * H)
            for k in range(KD):
                nc.vector.tensor_copy(
                    h1Tp[:, k, h * HL : (h + 1) * HL],
                    pairs(h1T)[:, k, h * HL : (h + 1) * HL],
                )
        rn1s_lo, rn1s_hi = rn1s_halves

        parts_act = pst.tile([P, M_TILES, G], FP32, tag="parts_act")
        parts_dve = pst.tile([P, M_TILES, G], FP32, tag="parts_dve")
        nc.gpsimd.memset(parts_act[:], 0.0)
        nc.gpsimd.memset(parts_dve[:], 0.0)

        # ---------- main loop over z2 groups ----------
        for g in range(G):
            nxt = {}
            for m in range(M_TILES):
                ps = ppsum.tile([P, PSUM_N], FP32, tag="ps")
                h2p = pairs(h2T_cur)
                for sub in range(NSUB):
                    nc.tensor.matmul(
                        ps[:, sub * 512 : (sub + 1) * 512],
                        h1Tp[:, :, m * P : (m + 1) * P],
                        h2p[:, :, sub * 512 : (sub + 1) * 512],
                        start=True,
                        stop=True,
                        perf_mode=DR,
                    )
                if m in DVE_MS:
                    nc.vector._custom_dve(
                        exp_op,
                        out=ps[:],
                        in0=ps[:],
                        s0=EXP_A,
                        s1=EXP_B,
                        imm2=EXP_C,
                        accum_out=parts_dve[:, m, g : g + 1],
                    )
                else:
                    nc.scalar.activation(
                        ps[:], ps[:], AF.Exp, scale=ACT_SCALE,
                        accum_out=parts_act[:, m, g : g + 1],
                    )
                if g + 1 < G:
                    if m == 0:
                        nxt["zt"] = load_group(g + 1)
                        sq_nxt = pgst.tile([P, G_TILES, D], BF16, tag="sq_scr")
                        ssq_nxt = pgst.tile([P, G_TILES], FP32, tag="ssqg")
                        hq_nxt = phq.tile([P, G_TILES, D], FP8, tag="hq")
                        h2T_nxt = ph2.tile([P, G_ROWS], BF16, tag="h2T")
                        nxt["sq"], nxt["ssq"] = sq_nxt, ssq_nxt
                        nxt["hq"], nxt["h2T"] = hq_nxt, h2T_nxt
                    elif m <= 8:
                        for t in range(2 * (m - 1), 2 * m):
                            if t % 2 == 0:
                                s = pscr.tile([P, D], FP32, tag="scr")
                                nc.scalar.activation(
                                    s[:], nxt["zt"][:, t, :], AF.Square,
                                    accum_out=nxt["ssq"][:, t : t + 1],
                                )
                            else:
                                sumsq(
                                    nxt["ssq"][:, t : t + 1], nxt["zt"][:, t, :]
                                )
                    elif m == 9:
                        nxt["rn32"] = rsqrt32_dve(
                            nxt["ssq"][:], pgst, "rng32", G_TILES
                        )
                    elif m in (10, 11, 12):
                        t0 = {10: 0, 11: 5, 12: 10}[m]
                        nt = {10: 5, 11: 5, 12: 6}[m]
                        for t in range(t0, t0 + nt):
                            nc.vector.tensor_scalar(
                                nxt["hq"][:, t, :], nxt["zt"][:, t, :],
                                nxt["rn32"][:, t : t + 1], None, ALU.mult,
                            )
                    elif m == 13:
                        xpose_burst2(nxt["hq"], 0, nxt["h2T"], 0, n=16)
                else:
                    # last group: the diagonal (positive-pair) path
                    if m == 0:
                        z2dt = pz2d.tile([P, M_TILES, D], FP32, tag="z2dt")
                        nc.sync.dma_start(
                            z2dt[:], z2d.rearrange("(t p) d -> p t d", p=P)
                        )
                    elif 6 <= m <= 9:
                        for t in range(4 * (m - 6), 4 * (m - 5)):
                            sumsq(ssq2d[:, t : t + 1], z2dt[:, t, :])
                    elif m == 10:
                        rn2d32 = rsqrt32_dve(ssq2d[:], pst, "rn2d32", M_TILES)
                    elif 11 <= m <= 14:
                        for mm in range(4 * (m - 11), 4 * (m - 10)):
                            s = pscr.tile([P, D], FP32, tag="scr")
                            nc.vector.scalar_tensor_tensor(
                                s[:],
                                in0=z1t[:, mm, :],
                                scalar=1.0,
                                in1=z2dt[:, mm, :],
                                op0=ALU.mult,
                                op1=ALU.mult,
                                accum_out=d_raw[:, mm : mm + 1],
                            )
            if g + 1 < G:
                zgt_cur = nxt["zt"]
                h2T_cur = nxt["h2T"]

        # ---------- finalize ----------
        st = pst.tile([P, M_TILES], FP32, tag="st")
        nc.vector.tensor_mul(st[:, :H], d_raw[:, :H], rn1s_lo[:])
        nc.vector.tensor_mul(st[:, H:], d_raw[:, H:], rn1s_hi[:])
        nc.vector.tensor_mul(st[:], st[:], rn2d32[:])
        nc.vector.tensor_scalar(st[:], st[:], 1.0 / (TAU * S2), None, ALU.mult)
        dex = pst.tile([P, M_TILES], FP32, tag="dex")
        nc.scalar.activation(dex[:], st[:], AF.Exp)
        rows_a = pst.tile([P, M_TILES], FP32, tag="rows_a")
        nc.vector.tensor_reduce(
            rows_a[:], parts_act[:], axis=mybir.AxisListType.X, op=ALU.add
        )
        rows_d = pst.tile([P, M_TILES], FP32, tag="rows_d")
        nc.vector.tensor_reduce(
            rows_d[:], parts_dve[:], axis=mybir.AxisListType.X, op=ALU.add
        )
        rows = pst.tile([P, M_TILES], FP32, tag="rows")
        nc.vector.scalar_tensor_tensor(
            rows[:], in0=rows_d[:], scalar=EXP_K, in1=rows_a[:],
            op0=ALU.mult, op1=ALU.add,
        )
        neg = pst.tile([P, M_TILES], FP32, tag="neg")
        nc.vector.tensor_sub(neg[:], rows[:], dex[:])
        lneg = pst.tile([P, M_TILES], FP32, tag="lneg")
        nc.scalar.activation(lneg[:], neg[:], AF.Ln)
        lp = pst.tile([P, M_TILES], FP32, tag="lp")
        nc.vector.tensor_sub(lp[:], lneg[:], st[:])
        nc.sync.dma_start(out_parts, lp[:])

    nc.compile()
    return nc


def get_nc():
    if "nc" not in _CACHE:
        _CACHE["nc"] = _build_nc()
    return _CACHE["nc"]


def make_in_maps(z1, z2):
    import ml_dtypes

    z1 = np.ascontiguousarray(np.asarray(z1, dtype=np.float32))
    z2 = np.ascontiguousarray(np.asarray(z2, dtype=np.float32))
    z2h = np.ascontiguousarray(z2.astype(ml_dtypes.bfloat16))
    in_maps = []
    for c in range(N_CORES):
        blk = slice(c * M_LOC, (c + 1) * M_LOC)
        in_maps.append({"z1": z1[blk], "z2": z2h, "z2d": z2[blk]})
    return in_maps


def kernel(z1, z2):
    from concourse.bass_utils import run_bass_kernel_spmd

    nc = get_nc()
    res = run_bass_kernel_spmd(nc, make_in_maps(z1, z2), core_ids=list(range(N_CORES)))
    total = 0.0
    for c in range(N_CORES):
        total += res.results[c]["loss_parts"].astype(np.float64).sum()
    return np.float32(total)
